# revision 2
# baseline (speedup 1.0000x reference)
"""Trainium2 Bass kernel for nn_CustomMHA (sparse head-gathered MHA), v2.

Two SPMD launches over 8 cores with free host-side reshuffle between them:

  L1 (projections, token-sharded): output heads are deduplicated on host
  into unique (source-head, input) combos: up to 15 normal heads sourced
  from x plus the special head sourced from x1/x2/x3. Duplicate output
  positions share attention entirely (their c_proj column blocks are
  summed on host). Each core projects q/k/v for all unique heads over its
  512-token chunk; weights packed 2 heads per 128-wide matmul block. All
  matmul inputs bf16 (same PE rate as fp32r, half the DMA). Bias-add +
  bf16 downcast runs on the otherwise-idle ACT engine.

  L2 (attention + c_proj, query-block-sharded): core c owns batch c//4,
  query block c%4 (512 queries) and runs full softmax attention for all
  unique heads over its block. ACT exp is the critical engine (cost is
  per-column): 74.7us/core, everything else is scheduled to hide under
  it. y is computed with a [v|ones] augmented matmul (row 64 = softmax
  denominator); the reciprocal row is broadcast across partitions with a
  1-row matmul (ones lhsT) instead of the slow pool broadcast + partition
  shift. Scores of head h+1 are emitted before y of head h so the
  in-order PE queue never starves while ACT drains exps; c_proj partials
  (per head pair, fp16, summed on host) fill remaining PE slack.
"""

import numpy as np
import ml_dtypes
from contextlib import ExitStack

import concourse.bass as bass
import concourse.tile as tile
from concourse import bacc, mybir
from concourse.bass_utils import run_bass_kernel_spmd

F32 = mybir.dt.float32
F32R = mybir.dt.float32r
BF16 = mybir.dt.bfloat16
F16 = mybir.dt.float16
AF = mybir.ActivationFunctionType
NPBF16 = ml_dtypes.bfloat16

B, T, C, H, D = 2, 2048, 1024, 16, 64
NC = 8
NT = B * T
TCH = NT // NC        # 512 tokens per core (both launches)
NCT = C // 128        # 8 contraction tiles
NKT = T // 128        # 16 key tiles per batch
SCALE = 1.0 / np.sqrt(D)


def _xi_of(j, nb):
    """Input selector for block j: 0 (=x) for normal blocks, 1/2/3
    (=x1/x2/x3) for the last three (special q/k/v) blocks."""
    return 0 if j < nb - 3 else j - (nb - 3) + 1


# ---------------------------------------------------------------- launch 1
def _build_l1(nb):
    nc = bacc.Bacc("TRN2", target_bir_lowering=False, debug=False,
                   num_devices=NC)
    XIN = nc.dram_tensor("XIN", [128, 4 * NCT * TCH], BF16,
                         kind="ExternalInput").ap()   # x|x1|x2|x3, ci-major
    WIN = nc.dram_tensor("WIN", [128, nb * NCT * 128], BF16,
                         kind="ExternalInput").ap()
    BIN = nc.dram_tensor("BIN", [128, nb], F32, kind="ExternalInput").ap()
    POUT = nc.dram_tensor("POUT", [128, nb * TCH], BF16,
                          kind="ExternalOutput").ap()

    with tile.TileContext(nc) as tc, ExitStack() as ctx:
        consts = ctx.enter_context(tc.tile_pool(name="consts", bufs=1))
        stg = ctx.enter_context(tc.tile_pool(name="stg", bufs=4))
        pp = ctx.enter_context(tc.tile_pool(name="pp", bufs=5, space="PSUM"))

        xin = consts.tile([128, 4 * NCT * TCH], BF16, tag="xin")
        win = consts.tile([128, nb * NCT * 128], BF16, tag="win")
        bin_ = consts.tile([128, nb], F32, tag="bin")

        XW = NCT * TCH       # cols per input chunk (4096)
        WB = NCT * 128       # cols per weight block (1024)
        # weights stream on the sync queue (FIFO) so later blocks' weights
        # always arrive ahead of need; x inputs stream FIFO on gpsimd with
        # x first (x1/x2/x3 are only read by the last three blocks)
        w1 = min(2, nb)
        nc.sync.dma_start(win[:, 0:w1 * WB], WIN[:, 0:w1 * WB])
        nc.gpsimd.dma_start(xin[:, 0:XW], XIN[:, 0:XW])
        nc.sync.dma_start(bin_[:], BIN[:])
        w2 = min(7, nb)
        if nb > w1:
            nc.sync.dma_start(win[:, w1 * WB:w2 * WB],
                              WIN[:, w1 * WB:w2 * WB])
        if nb > w2:
            nc.sync.dma_start(win[:, w2 * WB:], WIN[:, w2 * WB:])
        nc.gpsimd.dma_start(xin[:, XW:2 * XW], XIN[:, XW:2 * XW])
        nc.gpsimd.dma_start(xin[:, 2 * XW:3 * XW], XIN[:, 2 * XW:3 * XW])
        nc.gpsimd.dma_start(xin[:, 3 * XW:], XIN[:, 3 * XW:])

        for j in range(nb):
            xi = _xi_of(j, nb)
            ps = pp.tile([128, TCH], F32, tag="ps")
            for ci in range(NCT):
                nc.tensor.matmul(
                    ps[:],
                    win[:, j * WB + ci * 128:j * WB + (ci + 1) * 128],
                    xin[:, xi * XW + ci * TCH:xi * XW + (ci + 1) * TCH],
                    start=(ci == 0), stop=(ci == NCT - 1),
                )
            st = stg.tile([128, TCH], BF16, tag="st")
            with nc.allow_low_precision(reason="bf16 store of projections"):
                nc.scalar.activation(st[:], ps[:], AF.Identity,
                                     bias=bin_[:, j:j + 1])
            eng = nc.scalar if j % 2 == 0 else nc.gpsimd
            eng.dma_start(POUT[:, j * TCH:(j + 1) * TCH], st[:])

    nc.compile()
    return nc


# ---------------------------------------------------------------- launch 2
def _build_l2(nh):
    G = (nh + 1) // 2
    nc = bacc.Bacc("TRN2", target_bir_lowering=False, debug=False,
                   num_devices=NC)
    QT = nc.dram_tensor("QT", [64, nh * TCH], BF16, kind="ExternalInput").ap()
    KT = nc.dram_tensor("KT", [64, nh * T], BF16, kind="ExternalInput").ap()
    VA = nc.dram_tensor("VA", [128, nh * NKT * 65], BF16,
                        kind="ExternalInput").ap()
    WP = nc.dram_tensor("WP", [128, G * NCT * 128], BF16,
                        kind="ExternalInput").ap()
    ONES = nc.dram_tensor("ONES", [65, 64], F32R,
                          kind="ExternalInput").ap()
    OUT = nc.dram_tensor("OUT", [128, G * NCT * TCH], F16,
                         kind="ExternalOutput").ap()

    with tile.TileContext(nc) as tc, ExitStack() as ctx:
        consts = ctx.enter_context(tc.tile_pool(name="consts", bufs=1))
        exq = ctx.enter_context(tc.tile_pool(name="exq", bufs=12))
        nrm = ctx.enter_context(tc.tile_pool(name="nrm", bufs=3))
        ost = ctx.enter_context(tc.tile_pool(name="ost", bufs=4))
        pa = ctx.enter_context(tc.tile_pool(name="pa", bufs=2, space="PSUM"))
        py = ctx.enter_context(tc.tile_pool(name="py", bufs=2, space="PSUM"))
        pc = ctx.enter_context(tc.tile_pool(name="pc", bufs=2, space="PSUM"))

        qt = consts.tile([64, nh * TCH], BF16, tag="qt")
        kt = consts.tile([64, nh * T], BF16, tag="kt")
        va = consts.tile([128, nh * NKT * 65], BF16, tag="va")
        wp = consts.tile([128, G * NCT * 128], BF16, tag="wp")
        yn = consts.tile([128, G * TCH], BF16, tag="yn")
        ones = consts.tile([65, 64], F32R, tag="ones")

        VW = NKT * 65
        nc.sync.dma_start(qt[:, 0:TCH], QT[:, 0:TCH])
        nc.gpsimd.dma_start(kt[:, 0:T], KT[:, 0:T])
        if nh > 1:
            h2 = min(3, nh)
            nc.sync.dma_start(kt[:, T:h2 * T], KT[:, T:h2 * T])
        nc.gpsimd.dma_start(va[:, 0:VW], VA[:, 0:VW])
        nc.sync.dma_start(ones[:], ONES[:])
        if nh > 1:
            nc.sync.dma_start(qt[:, TCH:], QT[:, TCH:])
            nc.gpsimd.dma_start(va[:, VW:h2 * VW], VA[:, VW:h2 * VW])
            if nh > h2:
                nc.sync.dma_start(kt[:, h2 * T:], KT[:, h2 * T:])
                nc.gpsimd.dma_start(va[:, h2 * VW:], VA[:, h2 * VW:])
        nc.sync.dma_start(wp[:], WP[:])
        if nh % 2 == 1:
            nc.vector.memset(yn[64:128, (G - 1) * TCH:], 0.0)

        def scores(h):
            tiles = []
            for tp in range(NKT // 2):          # 8 psum tiles of 2 kt each
                ap_ = pa.tile([128, 1024], F32, tag="att")
                for k2 in range(2):
                    k = tp * 2 + k2
                    nc.tensor.matmul(
                        ap_[:, k2 * TCH:(k2 + 1) * TCH],
                        kt[:, h * T + k * 128:h * T + (k + 1) * 128],
                        qt[:, h * TCH:(h + 1) * TCH],
                        start=True, stop=True,
                    )
                tiles.append(ap_)
            return tiles

        def exps(h, tiles):
            exs = []
            for tp in range(NKT // 2):
                ex = exq.tile([128, 1024], BF16, tag="ex")
                with nc.allow_low_precision(reason="bf16 softmax weights"):
                    nc.scalar.activation(ex[:], tiles[tp][:], AF.Exp,
                                         scale=float(SCALE))
                exs.append(ex)
            return exs

        def ymm(h, exs):
            yp = py.tile([128, TCH], F32, tag="y")
            for k in range(NKT):
                nc.tensor.matmul(
                    yp[0:65, :],
                    va[:, (h * NKT + k) * 65:(h * NKT + k + 1) * 65],
                    exs[k // 2][:, (k % 2) * TCH:(k % 2 + 1) * TCH],
                    start=(k == 0), stop=(k == NKT - 1),
                )
            return yp

        def norm_a(h, yp):
            # copy y+sum to SBUF, reciprocal of the sum row (lane 64)
            ysb = nrm.tile([65, TCH], F32, tag="ysb")
            nc.vector.tensor_copy(ysb[:], yp[0:65, :])
            rc = nrm.tile([65, TCH], F32R, tag="rc")
            with nc.allow_low_precision(reason="f32 reciprocal"):
                nc.vector.reciprocal(rc[64:65, :], ysb[64:65, :])
            return ysb, rc

        def norm_b(h, ysb, rc):
            # broadcast 1/sum to 64 partitions via 1-row matmul, then
            # normalize into the yn pair slot (shift DMA for odd member)
            g, m = h // 2, h % 2
            bc = py.tile([128, TCH], F32, tag="y")
            nc.tensor.matmul(bc[0:64, :], ones[64:65, :], rc[64:65, :],
                             start=True, stop=True)
            with nc.allow_low_precision(reason="bf16 normalized y"):
                if m == 0:
                    nc.vector.tensor_mul(
                        yn[0:64, g * TCH:(g + 1) * TCH], ysb[0:64, :],
                        bc[0:64, :])
                else:
                    ys2 = nrm.tile([64, TCH], BF16, tag="ys2")
                    nc.vector.tensor_mul(ys2[:], ysb[0:64, :], bc[0:64, :])
                    nc.gpsimd.dma_start(
                        yn[64:128, g * TCH:(g + 1) * TCH], ys2[:])

        def cproj(g, act_split=False):
            for oi in range(NCT):
                cp = pc.tile([128, TCH], F32, tag="cp")
                nc.tensor.matmul(
                    cp[:],
                    wp[:, (g * NCT + oi) * 128:(g * NCT + oi + 1) * 128],
                    yn[:, g * TCH:(g + 1) * TCH],
                    start=True, stop=True,
                )
                st = ost.tile([128, TCH], F16, tag="ostg")
                with nc.allow_low_precision(reason="fp16 c_proj partial"):
                    # in the tail ACT is idle: alternate copies ACT/DVE
                    if act_split and oi % 2 == 0:
                        nc.scalar.copy(st[:], cp[:])
                    else:
                        nc.vector.tensor_copy(st[:], cp[:])
                eng = nc.sync if g % 2 == 0 else nc.gpsimd
                eng.dma_start(
                    OUT[:, (g * NCT + oi) * TCH:(g * NCT + oi + 1) * TCH],
                    st[:],
                )

        # software-pipelined emission; in the PE queue each iteration is
        # [scores(h) | bc(h-2) | y(h-1) | cproj(pair h-3)] so no matmul
        # ever waits on a freshly-emitted DVE/DMA dependency
        state = {}           # h -> (exs) or (ysb, rc)
        for h in range(nh + 3):
            if h < nh:
                at = scores(h)
                state[h] = ("exp", exps(h, at))
            if h - 2 >= 0 and h - 2 < nh:
                k, v = state[h - 2]
                if k == "nrm":
                    norm_b(h - 2, *v)
            if h - 1 >= 0 and h - 1 < nh:
                k, v = state[h - 1]
                yp = ymm(h - 1, v[0] if k == "nrm" else v)
                state[h - 1] = ("nrm", norm_a(h - 1, yp))
            if h - 3 >= 0 and (h - 3) % 2 == 1:
                g = (h - 3) // 2
                cproj(g, act_split=(g >= G - 2))
        if (nh - 1) % 2 == 0:
            cproj((nh - 1) // 2, act_split=True)

    nc.compile()
    return nc


_CACHE = {}


def _get_l1(nb):
    key = ("l1", nb)
    if key not in _CACHE:
        _CACHE[key] = _build_l1(nb)
    return _CACHE[key]


def _get_l2(nh):
    key = ("l2", nh)
    if key not in _CACHE:
        _CACHE[key] = _build_l2(nh)
    return _CACHE[key]


def _ci_major(a):
    """[C, cols] -> [128, NCT*cols] (ci-major packing of contraction dim)."""
    cols = a.shape[1]
    return np.ascontiguousarray(
        a.reshape(NCT, 128, cols).transpose(1, 0, 2).reshape(128, NCT * cols)
    )


def kernel(x1, x2, x3, x, W_attn, b_attn, W_proj, b_proj, head_idx, head_idxs,
           permutation):
    f32 = np.float32
    x1 = np.asarray(x1, f32).reshape(NT, C)
    x2 = np.asarray(x2, f32).reshape(NT, C)
    x3 = np.asarray(x3, f32).reshape(NT, C)
    x = np.asarray(x, f32).reshape(NT, C)
    W_attn = np.asarray(W_attn, f32)
    b_attn = np.asarray(b_attn, f32)
    W_proj = np.asarray(W_proj, f32)
    b_proj = np.asarray(b_proj, f32)
    hidx = int(head_idx)
    head_idxs = np.asarray(head_idxs).astype(np.int64)
    perm = np.asarray(permutation).astype(np.int64)

    Wq, Wk, Wv = W_attn[:C], W_attn[C:2 * C], W_attn[2 * C:]
    bq, bk, bv = b_attn[:C], b_attn[C:2 * C], b_attn[2 * C:]

    # dedup: output position -> source head; duplicates share attention and
    # their c_proj column blocks are summed
    special_pos = [i for i in range(H) if perm[i] == 0]
    normal = [(i, int(head_idxs[perm[i] - 1])) for i in range(H) if perm[i] != 0]
    uniq = sorted(set(s for _, s in normal))
    wp_map = {u: np.zeros((C, D), f32) for u in uniq}
    for i, s in normal:
        wp_map[s] += W_proj[:, i * D:(i + 1) * D]
    wp_s = np.zeros((C, D), f32)
    for i in special_pos:
        wp_s += W_proj[:, i * D:(i + 1) * D]

    nU = len(uniq)
    NH = nU + 1           # unique heads incl. special (last)
    G = (NH + 1) // 2

    # ---------------- L1 host prep
    cols = []
    for u in uniq:
        s = slice(u * D, (u + 1) * D)
        cols.append((Wq[s], bq[s]))
        cols.append((Wk[s], bk[s]))
        cols.append((Wv[s], bv[s]))
    ncols_norm = len(cols)
    nb_norm = (ncols_norm + 1) // 2
    s = slice(hidx * D, (hidx + 1) * D)
    special_cols = [(Wq[s], bq[s]), (Wk[s], bk[s]), (Wv[s], bv[s])]
    nb = nb_norm + 3

    WIN = np.zeros((128, nb * NCT * 128), NPBF16)
    BIN = np.zeros((128, nb), f32)
    for j in range(nb):
        blk = np.zeros((C, 128), f32)
        bias = np.zeros(128, f32)
        if j < nb_norm:
            members = cols[2 * j:2 * j + 2]
        else:
            members = [special_cols[j - nb_norm]]
        for m, (w, b) in enumerate(members):
            blk[:, m * D:(m + 1) * D] = w.T
            bias[m * D:(m + 1) * D] = b
        WIN[:, j * NCT * 128:(j + 1) * NCT * 128] = _ci_major(
            blk.astype(NPBF16))
        BIN[:, j] = bias

    xs = [x, x1, x2, x3]
    in_maps1 = []
    for c in range(NC):
        tok = slice(c * TCH, (c + 1) * TCH)
        xin = np.concatenate(
            [_ci_major(np.ascontiguousarray(xi[tok].T).astype(NPBF16))
             for xi in xs], axis=1)
        in_maps1.append({"XIN": xin, "WIN": WIN, "BIN": BIN})

    res1 = run_bass_kernel_spmd(_get_l1(nb), in_maps1, list(range(NC))).results

    # ---------------- reshuffle: per-core POUT -> per (head, proj) over NT
    qkv = np.zeros((NH, 3, 64, NT), NPBF16)
    for c in range(NC):
        p = res1[c]["POUT"]           # [128, nb*TCH] bf16
        tok = slice(c * TCH, (c + 1) * TCH)
        for ci in range(ncols_norm):
            j, m = ci // 2, ci % 2
            qkv[ci // 3, ci % 3, :, tok] = \
                p[m * D:(m + 1) * D, j * TCH:(j + 1) * TCH]
        for sj in range(3):
            j = nb_norm + sj
            qkv[NH - 1, sj, :, tok] = p[0:D, j * TCH:(j + 1) * TCH]

    # ---------------- L2 host prep
    WPg = np.zeros((128, G * NCT * 128), NPBF16)
    wlist = [wp_map[u] for u in uniq] + [wp_s]
    for h in range(NH):
        g, m = h // 2, h % 2
        wT = wlist[h].T.astype(NPBF16)       # [D, C]
        for oi in range(NCT):
            WPg[m * D:(m + 1) * D,
                (g * NCT + oi) * 128:(g * NCT + oi + 1) * 128] = \
                wT[:, oi * 128:(oi + 1) * 128]

    in_maps2 = []
    for c in range(NC):
        b = c // (NC // B)
        bt = slice(b * T, (b + 1) * T)
        tok = slice(c * TCH, (c + 1) * TCH)
        QTm = np.concatenate([qkv[h, 0][:, tok] for h in range(NH)], axis=1)
        KTm = np.concatenate([qkv[h, 1][:, bt] for h in range(NH)], axis=1)
        VAm = np.zeros((128, NH * NKT * 65), NPBF16)
        for h in range(NH):
            vb = qkv[h, 2][:, bt]            # [64, T]
            vkt = np.ascontiguousarray(vb.T).reshape(NKT, 128, 64)
            blk = np.zeros((NKT, 128, 65), NPBF16)
            blk[:, :, 64] = NPBF16(1.0)
            blk[:, :, 0:64] = vkt
            VAm[:, h * NKT * 65:(h + 1) * NKT * 65] = \
                blk.transpose(1, 0, 2).reshape(128, NKT * 65)
        in_maps2.append({
            "QT": np.ascontiguousarray(QTm),
            "KT": np.ascontiguousarray(KTm),
            "VA": VAm, "WP": WPg,
            "ONES": np.ones((65, 64), np.float32),
        })

    res2 = run_bass_kernel_spmd(_get_l2(NH), in_maps2, list(range(NC))).results

    # ---------------- assemble output
    out = np.zeros((NT, C), np.float64)
    for c in range(NC):
        o = res2[c]["OUT"].astype(np.float64)   # [128, G*NCT*TCH]
        tok = slice(c * TCH, (c + 1) * TCH)
        blk = np.zeros((C, TCH), np.float64)
        for g in range(G):
            for oi in range(NCT):
                blk[oi * 128:(oi + 1) * 128] += \
                    o[:, (g * NCT + oi) * TCH:(g * NCT + oi + 1) * TCH]
        out[tok] = blk.T
    out = out.astype(f32) + b_proj[None, :]
    return out.reshape(B, T, C).astype(f32)
